# revision 1
# baseline (speedup 1.0000x reference)
"""CTC loss (nn_CTC_28819230556189) on 8 Trainium2 NeuronCores via Bass/Tile.

Strategy (data-parallel over batch, 4 examples per core):
  - logits = hpad @ W.T + b ; log_softmax over V=5000 ; CTC forward DP.
  - Only two reductions of the [T,V] logits are needed per (b,t):
      lse[b,t]  = logsumexp_V(logits)   (constant-shift trick, no row max)
      glog[b,t,s] = logits[b, t, ext[b,s]]  via a small matmul with
      host-gathered W[ext[b]] columns (avoids any on-device gather).
  - DP runs in the linear domain with renormalization every 8 steps:
      alpha' = (alpha + alpha>>1 + allow2 * (alpha>>2)) * p_t,
      p_t = exp(glog - lse) in bf16, alpha in bf16, [4ex, 201] per core.
  - loss partials summed on host (no collectives needed).

Host-side prep (transposes / gathers / bf16 casts) is numpy; all FLOPs of
consequence (matmul, softmax, DP) run on the NeuronCores.
"""

import numpy as np
import ml_dtypes

import concourse.bass as bass
import concourse.bacc as bacc
import concourse.tile as tile
import concourse.mybir as mybir
from concourse.bass_utils import run_bass_kernel_spmd

BF16 = mybir.dt.bfloat16
F32 = mybir.dt.float32
AF = mybir.ActivationFunctionType
ALU = mybir.AluOpType
AX = mybir.AxisListType

# Problem shapes (hardcoded per spec nn_CTC_28819230556189)
B, T, E, V, L = 32, 500, 1024, 5000, 100
S = 2 * L + 1          # 201 extended labels
NCORE = 8
BPC = B // NCORE       # 4 examples per core
NE = E // 128          # 8 contraction tiles
TC = 125               # time chunk
NCHUNK = T // TC       # 4
VC = 500               # v-chunk width (one PSUM bank in f32)
NV = V // VC           # 10
C_SHIFT = 4.0          # logsumexp constant shift (logits ~ N(0,1))
SC = 9.0               # per-step scale exp(SC) folded into p to keep alpha ~O(1)
RENORM = 16            # renormalize alpha every RENORM steps
NR = (T - 1) // RENORM + 2  # renorm slots (31 used)

_cache = {}


def _build_nc():
    nc = bacc.Bacc("TRN2", target_bir_lowering=False, debug=False,
                   enable_asserts=False)

    # register const APs used as activation biases
    for val in (-C_SHIFT, SC - C_SHIFT):
        _cth = nc.alloc_sbuf_tensor(f"const-f32-{val}", [128, 1], F32)
        nc.gpsimd.memset(_cth.ap(), val)
        nc.const_aps.aps[(F32, val)] = _cth.ap()
    nc.all_engine_barrier()

    hpt_d = nc.dram_tensor("hpt", [BPC, NE, 128, T], BF16, kind="ExternalInput")
    wtt_d = nc.dram_tensor("wtt", [NE, 128, V], BF16, kind="ExternalInput")
    wext_d = nc.dram_tensor("wext", [BPC, NE, 128, S], BF16, kind="ExternalInput")
    m2_d = nc.dram_tensor("m2", [BPC, S], BF16, kind="ExternalInput")
    out_d = nc.dram_tensor("out", [1, 1], F32, kind="ExternalOutput")

    with tile.TileContext(nc) as tc:
      with tc.tile_pool(name="persist", bufs=1) as pers:
        def ptile(shape, dtype, nm):
            return pers.tile(shape, dtype, tag=nm, name=nm)
        # ---- resident weights ----
        wt_all = ptile([128, NE * V], BF16, "wt_all")
        for e in range(NE):
            nc.sync.dma_start(wt_all[:, e * V:(e + 1) * V], wtt_d[e])
        wext_all = ptile([128, BPC * NE * S], BF16, "wext_all")
        for bb in range(BPC):
            for e in range(NE):
                nc.sync.dma_start(
                    wext_all[:, (bb * NE + e) * S:(bb * NE + e + 1) * S],
                    wext_d[bb, e])
        m2t = ptile([BPC, S], BF16, "m2t")
        nc.sync.dma_start(m2t[:], m2_d[:])

        # ---- DP state (persistent) ----
        # alpha[s] lives at col s+2; cols 0,1 are a zero halo for the shifts.
        A0 = ptile([BPC, S + 2], BF16, "A0")
        A1 = ptile([BPC, S + 2], BF16, "A1")
        nc.vector.memset(A0[:], 0.0)
        nc.vector.memset(A1[:], 0.0)
        t1 = ptile([BPC, S], BF16, "t1")
        t2 = ptile([BPC, S], BF16, "t2")
        t3 = ptile([BPC, S], BF16, "t3")
        R = ptile([BPC, NR], F32, "R")
        nc.vector.memset(R[:], 1.0)
        rinv = ptile([BPC, 1], F32, "rinv")

        with (
            tc.tile_pool(name="hp", bufs=2) as hp_pool,
            tc.tile_pool(name="ps", bufs=4, space="PSUM") as ps_pool,
            tc.tile_pool(name="glog", bufs=2, space="PSUM") as glog_pool,
            tc.tile_pool(name="scr", bufs=3) as scr_pool,
            tc.tile_pool(name="small", bufs=4) as small_pool,
            tc.tile_pool(name="pch", bufs=2) as p_pool,
            tc.tile_pool(name="pb", bufs=2) as pb_pool,
        ):
            cur = A0
            for c in range(NCHUNK):
                t0 = c * TC
                p_t = p_pool.tile([TC, BPC, S], BF16, tag="p")
                for bb in range(BPC):
                    hp_t = hp_pool.tile([128, NE * TC], BF16, tag="hp")
                    for e in range(NE):
                        nc.sync.dma_start(hp_t[:, e * TC:(e + 1) * TC],
                                          hpt_d[bb, e, :, t0:t0 + TC])
                    spart = small_pool.tile([TC, NV], F32, tag="spart")
                    for v in range(NV):
                        ps = ps_pool.tile([TC, VC], F32, tag="ps")
                        for e in range(NE):
                            nc.tensor.matmul(
                                ps[:],
                                hp_t[:, e * TC:(e + 1) * TC],
                                wt_all[:, e * V + v * VC: e * V + (v + 1) * VC],
                                start=(e == 0), stop=(e == NE - 1))
                        scr = scr_pool.tile([TC, VC], BF16, tag="scr")
                        nc.scalar.activation(scr[:], ps[:], AF.Exp,
                                             bias=-C_SHIFT, scale=1.0,
                                             accum_out=spart[:, v:v + 1])
                    glog = glog_pool.tile([TC, S], F32, tag="glog")
                    for e in range(NE):
                        nc.tensor.matmul(
                            glog[:],
                            hp_t[:, e * TC:(e + 1) * TC],
                            wext_all[:, (bb * NE + e) * S:(bb * NE + e + 1) * S],
                            start=(e == 0), stop=(e == NE - 1))
                    scr10 = small_pool.tile([TC, NV], BF16, tag="scr10")
                    lsum = small_pool.tile([TC, 1], F32, tag="lsum")
                    nc.scalar.activation(scr10[:], spart[:], AF.Identity,
                                         accum_out=lsum[:])
                    nbias = small_pool.tile([TC, 1], F32, tag="nbias")
                    nc.scalar.activation(nbias[:], lsum[:], AF.Ln)
                    # nbias = -(ln lsum) - C + SC  ->  p = exp(glog - lse + SC)
                    nc.scalar.activation(nbias[:], nbias[:], AF.Identity,
                                         scale=-1.0, bias=SC - C_SHIFT)
                    nc.scalar.activation(p_t[:, bb, :], glog[:], AF.Exp,
                                         bias=nbias[:], scale=1.0)
                # flatten p [t, b, s] -> [b, t*s] for the DP
                PB = pb_pool.tile([BPC, TC * S], BF16, tag="pb")
                for bb in range(BPC):
                    nc.sync.dma_start(PB[bb:bb + 1, :], p_t[:, bb, :])
                # ---- DP steps for this chunk ----
                for tl in range(TC):
                    t = t0 + tl
                    pc = PB[:, tl * S:(tl + 1) * S]
                    if t == 0:
                        nc.vector.tensor_copy(cur[:, 2:4], pc[:, 0:2])
                        continue
                    prv, cur = cur, (A1 if cur is A0 else A0)
                    nc.vector.tensor_add(t1[:], prv[:, 1:S + 1], prv[:, 2:S + 2])
                    nc.vector.tensor_mul(t2[:], prv[:, 0:S], m2t[:])
                    nc.vector.tensor_add(t3[:], t1[:], t2[:])
                    nc.vector.tensor_mul(cur[:, 2:S + 2], t3[:], pc)
                    if t % RENORM == RENORM - 1:
                        k = t // RENORM
                        nc.vector.tensor_reduce(R[:, k:k + 1], cur[:, 2:S + 2],
                                                axis=AX.X, op=ALU.add)
                        nc.vector.reciprocal(rinv[:], R[:, k:k + 1])
                        nc.vector.tensor_scalar_mul(cur[:, 2:S + 2],
                                                    cur[:, 2:S + 2], rinv[:])

            # ---- finalize: ll = ln(a[-1]+a[-2]) + sum ln(r) ----
            u = ptile([BPC, 1], F32, "u")
            nc.vector.tensor_add(u[:], cur[:, S:S + 1], cur[:, S + 1:S + 2])
            lnu = ptile([BPC, 1], F32, "lnu")
            nc.scalar.activation(lnu[:], u[:], AF.Ln)
            rlog = ptile([BPC, NR], F32, "rlog")
            nc.scalar.activation(rlog[:], R[:], AF.Ln)
            rs = ptile([BPC, 1], F32, "rs")
            nc.vector.tensor_reduce(rs[:], rlog[:], axis=AX.X, op=ALU.add)
            llv = ptile([BPC, 1], F32, "llv")
            nc.vector.tensor_add(llv[:], lnu[:], rs[:])
            llf = ptile([1, BPC], F32, "llf")
            nc.sync.dma_start(llf[:], llv[:])  # [4,1] -> [1,4] partition flatten
            tot = ptile([1, 1], F32, "tot")
            nc.vector.tensor_reduce(tot[:], llf[:], axis=AX.X, op=ALU.add)
            nc.sync.dma_start(out_d[:], tot[:])

    nc.compile()
    return nc


def kernel(hpad, W, b, ys):
    assert hpad.shape == (B, T, E) and W.shape == (V, E) and ys.shape == (B, L)
    assert not np.any(np.asarray(b)), "kernel assumes b == 0 (per problem spec)"

    # ---- host prep ----
    ext = np.zeros((B, S), dtype=np.int64)
    ext[:, 1::2] = ys
    prev2 = np.full((B, S), -1, dtype=np.int64)
    prev2[:, 2:] = ext[:, :-2]
    allow2 = ((ext != 0) & (ext != prev2)).astype(ml_dtypes.bfloat16)

    hpT = np.ascontiguousarray(hpad.transpose(0, 2, 1)).astype(ml_dtypes.bfloat16)
    hpT = hpT.reshape(B, NE, 128, T)
    wtT = np.ascontiguousarray(W.T).astype(ml_dtypes.bfloat16).reshape(NE, 128, V)
    # W[ext[b]].T : [B, E, S] -> [B, NE, 128, S]
    wext = np.ascontiguousarray(
        W[ext.reshape(-1)].reshape(B, S, E).transpose(0, 2, 1)
    ).astype(ml_dtypes.bfloat16).reshape(B, NE, 128, S)

    if "nc" not in _cache:
        _cache["nc"] = _build_nc()
    nc = _cache["nc"]

    in_maps = []
    for c in range(NCORE):
        sl = slice(c * BPC, (c + 1) * BPC)
        in_maps.append({
            "hpt": np.ascontiguousarray(hpT[sl]),
            "wtt": wtT,
            "wext": np.ascontiguousarray(wext[sl]),
            "m2": np.ascontiguousarray(allow2[sl]),
        })

    res = run_bass_kernel_spmd(nc, in_maps, core_ids=list(range(NCORE)))
    tot = sum(float(r["out"][0, 0]) for r in res.results)
    tot -= B * T * SC  # undo the per-step exp(SC) scaling of p
    return np.float32(-tot / B)



# revision 7
# speedup vs baseline: 1.2368x; 1.2368x over previous
"""CTC loss (nn_CTC_28819230556189) on 8 Trainium2 NeuronCores via Bass/Tile.

Data-parallel over batch (4 examples/core). Per core:

  Phase 1 (PE + Act):  logits = hpad @ W.T in fp8 DoubleRow (K=256/matmul);
    lse side:  exp(logit - C) accumulated over V -> lsum[t]; ln(lsum)
               partition-reduced via a ones-matmul -> Sum_t ln lsum (llacc).
    glog side: glog^T[s,t] = (W[ext] )^T @ h in fp8 -> p~ = exp(glog + D)
               written as fp8 tiles [s,t], DMA-transposed into
               psweep[example, s, t].
    The -lse term is NOT folded into p~: every CTC path takes exactly one
    emission per frame, so ll = ln(sum B~) - Sum_t lse_t - T*D, with
    lse_t = ln lsum_t + C.  (Avoids all per-chunk bias plumbing.)

  Phase 2 (DVE): CTC forward DP restructured as an s-sweep: for each
    extended-label state s (201 of them), ONE tensor_tensor_scan over all
    T=500 frames computes  B_s(t) = (B_s(t-1) + v_s(t)) * p~_t[s]  with
    v_s = B_{s-1}(t-1) [+ B_{s-2}(t-1) at non-blank s, masked only at the
    rare repeated-label positions].  ~300 DVE ops total vs ~2100 for the
    per-frame formulation; scan state is fp32 internally.

  Loss partials summed on host (no collectives needed).
"""

import numpy as np

import concourse.bass as bass
import concourse.bacc as bacc
import concourse.tile as tile
import concourse.mybir as mybir
from concourse.bass_utils import run_bass_kernel_spmd

BF16 = mybir.dt.bfloat16
F32 = mybir.dt.float32
FP8 = mybir.dt.float8e4
AF = mybir.ActivationFunctionType
ALU = mybir.AluOpType
AX = mybir.AxisListType
DR = mybir.MatmulPerfMode.DoubleRow

# Problem shapes (hardcoded per spec nn_CTC_28819230556189)
B, T, E, V, L = 32, 500, 1024, 5000, 100
S = 2 * L + 1           # 201 extended labels
NCORE = 8
BPC = B // NCORE        # 4 examples per core
NPAIR = E // 256        # 4 double-row K-pairs (256 contraction each)
TC = 125                # time chunk
NCHUNK = T // TC        # 4
VC = 500                # v-chunk width (one PSUM bank in f32)
NV = V // VC            # 10
C_SHIFT = 4.0           # logsumexp constant shift (logits ~ N(0,1))
D_SHIFT = -1.1          # p~ = exp(glog + D); keeps ln(sum B~) drift ~ 0
S0 = 128                # first s-tile width (second is S - S0 = 73)
TCP = 128               # hp stationary pair-stride (16B-aligned pad of TC)
SP = 208                # wext stationary pair-stride (16B-aligned pad of S)

_cache = {}


def _build_nc(masked_odd):
    """masked_odd: sorted tuple of odd s positions where some example in the
    batch has a repeated label (skip transition disallowed) -> those sweep
    iterations apply the per-example m2 mask; all other odd s use a plain
    add (mask == 1 for every example by construction)."""
    nc = bacc.Bacc("TRN2", target_bir_lowering=False, debug=False,
                   enable_asserts=False)

    for val in (-C_SHIFT, D_SHIFT):
        cth = nc.alloc_sbuf_tensor(f"const-f32-{val}", [128, 1], F32)
        nc.gpsimd.memset(cth.ap(), val)
        nc.const_aps.aps[(F32, val)] = cth.ap()
    nc.all_engine_barrier()

    hpt_d = nc.dram_tensor("hpt", [BPC, NPAIR, 2, 128, T], FP8,
                           kind="ExternalInput")
    wtt_d = nc.dram_tensor("wtt", [NPAIR, 2, 128, V], FP8,
                           kind="ExternalInput")
    wxt_d = nc.dram_tensor("wxt", [BPC, NPAIR, 2, 128, S], FP8,
                           kind="ExternalInput")
    m2_d = nc.dram_tensor("m2", [BPC, S], F32, kind="ExternalInput")
    out_d = nc.dram_tensor("out", [1, 1], F32, kind="ExternalOutput")

    with tile.TileContext(nc) as tc:
      with tc.tile_pool(name="persist", bufs=1) as pers:
        def ptile(shape, dtype, nm):
            return pers.tile(shape, dtype, tag=nm, name=nm)

        # ---- resident weights ----
        wt_all = ptile([128, NPAIR, 2, V], FP8, "wt_all")
        for pe in range(NPAIR):
            for i in range(2):
                nc.sync.dma_start(wt_all[:, pe, i, :], wtt_d[pe, i])
        wx_all = ptile([128, BPC, NPAIR, 2, SP], FP8, "wx_all")
        for bb in range(BPC):
            for pe in range(NPAIR):
                for i in range(2):
                    nc.sync.dma_start(wx_all[:, bb, pe, i, 0:S],
                                      wxt_d[bb, pe, i])
        m2t = ptile([BPC, S], F32, "m2t")
        nc.sync.dma_start(m2t[:], m2_d[:])
        ones125 = ptile([125, 1], BF16, "ones125")
        nc.vector.memset(ones125[:], 1.0)

        # ---- sweep state ----
        psweep = ptile([BPC, S, T], FP8, "psweep")
        brows = ptile([BPC, 3, T + 1], BF16, "brows")
        nc.vector.memset(brows[:], 0.0)
        nc.vector.memset(brows[:, 0, 0:1], 1.0)   # B_0(-1) = 1
        zrow = ptile([BPC, T], BF16, "zrow")
        nc.vector.memset(zrow[:], 0.0)
        vtmp = ptile([BPC, T], BF16, "vtmp")
        llacc = ptile([1, BPC], F32, "llacc")     # Sum_t ln lsum_t per ex
        nc.vector.memset(llacc[:], 0.0)

        with (
            tc.tile_pool(name="hp", bufs=2) as hp_pool,
            tc.tile_pool(name="scr", bufs=2) as scr_pool,
            tc.tile_pool(name="small", bufs=4) as small_pool,
            tc.tile_pool(name="pt", bufs=2) as pt_pool,
            tc.tile_pool(name="ps", bufs=4, space="PSUM") as ps_pool,
            tc.tile_pool(name="gl0", bufs=1, space="PSUM") as gl0_pool,
            tc.tile_pool(name="gl1", bufs=1, space="PSUM") as gl1_pool,
            tc.tile_pool(name="lsps", bufs=2, space="PSUM") as lsps_pool,
        ):
            # ================= Phase 1: matmuls / lse / p~ =================
            for c in range(NCHUNK):
                t0 = c * TC
                for bb in range(BPC):
                    hp_t = hp_pool.tile([128, NPAIR, 2, TCP], FP8, tag="hp",
                                        name="hp_t")
                    for pe in range(NPAIR):
                        for i in range(2):
                            nc.sync.dma_start(
                                hp_t[:, pe, i, 0:TC],
                                hpt_d[bb, pe, i, :, t0:t0 + TC])

                    spart = small_pool.tile([TC, NV], F32, tag="spart",
                                            name="spart")
                    for v in range(NV):
                        ps = ps_pool.tile([TC, VC], F32, tag="ps", name="ps")
                        for pe in range(NPAIR):
                            nc.tensor.matmul(
                                ps[:],
                                hp_t[:, pe, :, 0:TC],
                                wt_all[:, pe, :, v * VC:(v + 1) * VC],
                                start=(pe == 0), stop=(pe == NPAIR - 1),
                                perf_mode=DR)
                        scr = scr_pool.tile([TC, VC], BF16, tag="scr",
                                            name="scr")
                        nc.scalar.activation(scr[:], ps[:], AF.Exp,
                                             bias=-C_SHIFT, scale=1.0,
                                             accum_out=spart[:, v:v + 1])
                    scr10 = small_pool.tile([TC, NV], BF16, tag="scr10",
                                            name="scr10")
                    lsum = small_pool.tile([TC, 1], F32, tag="lsum",
                                           name="lsum")
                    nc.scalar.activation(scr10[:], spart[:], AF.Identity,
                                         accum_out=lsum[:])
                    lnls = small_pool.tile([TC, 1], BF16, tag="lnls",
                                           name="lnls")
                    nc.scalar.activation(lnls[:], lsum[:], AF.Ln)
                    # Sum_t ln lsum via ones-matmul partition reduce
                    lsps = lsps_pool.tile([1, 1], F32, tag="lsps",
                                          name="lsps")
                    nc.tensor.matmul(lsps[:], ones125[:], lnls[:],
                                     start=True, stop=True)
                    nc.vector.tensor_add(llacc[:, bb:bb + 1],
                                         llacc[:, bb:bb + 1], lsps[:])

                    # glog^T  [s, t] tiles, then p~ = exp(glog + D) in fp8
                    gl0 = gl0_pool.tile([S0, TC], F32, tag="gl0", name="gl0")
                    gl1 = gl1_pool.tile([S - S0, TC], F32, tag="gl1",
                                        name="gl1")
                    for pe in range(NPAIR):
                        nc.tensor.matmul(
                            gl0[:], wx_all[:, bb, pe, :, 0:S0],
                            hp_t[:, pe, :, 0:TC],
                            start=(pe == 0), stop=(pe == NPAIR - 1),
                            perf_mode=DR)
                    for pe in range(NPAIR):
                        nc.tensor.matmul(
                            gl1[:], wx_all[:, bb, pe, :, S0:S],
                            hp_t[:, pe, :, 0:TC],
                            start=(pe == 0), stop=(pe == NPAIR - 1),
                            perf_mode=DR)
                    pt0 = pt_pool.tile([S0, TC], FP8, tag="pt0", name="pt0")
                    pt1 = pt_pool.tile([S - S0, TC], FP8, tag="pt1",
                                       name="pt1")
                    nc.scalar.activation(pt0[:], gl0[:], AF.Exp,
                                         bias=D_SHIFT, scale=1.0)
                    nc.scalar.activation(pt1[:], gl1[:], AF.Exp,
                                         bias=D_SHIFT, scale=1.0)
                    nc.sync.dma_start(psweep[bb:bb + 1, 0:S0, t0:t0 + TC],
                                      pt0[:])
                    nc.sync.dma_start(psweep[bb:bb + 1, S0:S, t0:t0 + TC],
                                      pt1[:])

            # ================= Phase 2: s-sweep DP =================
            masked = set(masked_odd)
            for s in range(S):
                row = brows[:, s % 3, :]
                out_ap = row[:, 1:T + 1]
                p_s = psweep[:, s, :]
                if s == 3:
                    # B_0(-1)=1 was consumed by s=1; row 0 now recycles as
                    # B_3 whose halo must read 0 for s=4/s=5.
                    nc.vector.memset(brows[:, 0, 0:1], 0.0)
                if s == 0:
                    nc.vector.tensor_tensor_scan(
                        out_ap, zrow[:], p_s, 1.0, ALU.add, ALU.mult)
                    continue
                b1 = brows[:, (s - 1) % 3, 0:T]
                if s == 1 or s % 2 == 0:
                    # v = B_{s-1}(t-1) only (blank, or s=1 which has no s-2)
                    nc.vector.tensor_tensor_scan(
                        out_ap, b1, p_s, 0.0, ALU.add, ALU.mult)
                    continue
                b2 = brows[:, (s - 2) % 3, 0:T]
                if s in masked:
                    nc.vector.tensor_scalar_mul(vtmp[:], b2,
                                                m2t[:, s:s + 1])
                    nc.vector.tensor_add(vtmp[:], vtmp[:], b1)
                else:
                    nc.vector.tensor_add(vtmp[:], b1, b2)
                nc.vector.tensor_tensor_scan(
                    out_ap, vtmp[:], p_s, 0.0, ALU.add, ALU.mult)

            # ================= finalize =================
            u = ptile([BPC, 1], F32, "u")
            nc.vector.tensor_add(u[:], brows[:, 200 % 3, T:T + 1],
                                 brows[:, 199 % 3, T:T + 1])
            lnu = ptile([BPC, 1], F32, "lnu")
            nc.scalar.activation(lnu[:], u[:], AF.Ln)
            llf = ptile([1, BPC], F32, "llf")
            nc.sync.dma_start(llf[:], lnu[:])   # [4,1] -> [1,4]
            dif = ptile([1, BPC], F32, "dif")
            nc.vector.tensor_tensor(dif[:], llf[:], llacc[:], ALU.subtract)
            tot = ptile([1, 1], F32, "tot")
            nc.vector.tensor_reduce(tot[:], dif[:], axis=AX.X, op=ALU.add)
            nc.sync.dma_start(out_d[:], tot[:])

    nc.compile()
    return nc


def prep_in_maps(hpad, W, b, ys):
    """Host-side layout prep shared by kernel() and test harnesses."""
    f8 = mybir.dt.np(FP8)
    ext = np.zeros((B, S), dtype=np.int64)
    ext[:, 1::2] = ys
    prev2 = np.full((B, S), -1, dtype=np.int64)
    prev2[:, 2:] = ext[:, :-2]
    allow2 = (ext != 0) & (ext != prev2)
    masked_odd = tuple(sorted(
        s for s in range(3, S, 2) if not allow2[:, s].all()))
    m2 = allow2.astype(np.float32)

    hpT = np.ascontiguousarray(hpad.transpose(0, 2, 1)).astype(f8)
    hpT = hpT.reshape(B, NPAIR, 2, 128, T)
    wtT = np.ascontiguousarray(np.asarray(W).T).astype(f8)
    wtT = wtT.reshape(NPAIR, 2, 128, V)
    wext = np.ascontiguousarray(
        np.asarray(W)[ext.reshape(-1)].reshape(B, S, E).transpose(0, 2, 1)
    ).astype(f8).reshape(B, NPAIR, 2, 128, S)

    in_maps = []
    for c in range(NCORE):
        sl = slice(c * BPC, (c + 1) * BPC)
        in_maps.append({
            "hpt": np.ascontiguousarray(hpT[sl]),
            "wtt": wtT,
            "wxt": np.ascontiguousarray(wext[sl]),
            "m2": np.ascontiguousarray(m2[sl]),
        })
    return in_maps, masked_odd


def kernel(hpad, W, b, ys):
    assert hpad.shape == (B, T, E) and W.shape == (V, E) and ys.shape == (B, L)
    assert not np.any(np.asarray(b)), "kernel assumes b == 0 (per problem spec)"

    in_maps, masked_odd = prep_in_maps(hpad, W, b, ys)
    key = ("nc", masked_odd)
    if key not in _cache:
        _cache[key] = _build_nc(masked_odd)
    nc = _cache[key]
    _cache["nc_last"] = (nc, in_maps)

    res = run_bass_kernel_spmd(nc, in_maps, core_ids=list(range(NCORE)))
    tot = sum(float(r["out"][0, 0]) for r in res.results)
    ll_sum = tot - B * T * (C_SHIFT + D_SHIFT)
    return np.float32(-ll_sum / B)


# revision 8
# speedup vs baseline: 1.2847x; 1.0387x over previous
"""CTC loss (nn_CTC_28819230556189) on 8 Trainium2 NeuronCores via Bass/Tile.

Data-parallel over batch (4 examples/core). Per core:

  Phase 1 (PE + Act):  logits = hpad @ W.T in fp8 DoubleRow (K=256/matmul);
    lse side:  exp(logit - C) accumulated over V -> lsum[t]; ln(lsum)
               partition-reduced via a ones-matmul -> Sum_t ln lsum (llacc).
    glog side: the extended label sequence has only 101 distinct tokens per
               example (blank + 100 labels), so glog^T is computed as a
               [101, t] matmul; p~ = exp(glog + D) in bf16 is DMA-transposed
               into psweep[example, row, t] (row 0 = blank, row 1+j = label j).
    The -lse term is NOT folded into p~: every CTC path takes exactly one
    emission per frame, so ll = ln(sum B~) - Sum_t lse_t - T*D, with
    lse_t = ln lsum_t + C.

  Phase 2 (DVE): CTC forward DP restructured as an s-sweep: for each
    extended-label state s (201 of them), ONE tensor_tensor_scan over the
    frames computes  B_s(t) = (B_s(t-1) + v_s(t)) * p~_t[s]  with
    v_s = B_{s-1}(t-1) [+ B_{s-2}(t-1) at non-blank s, masked only at the
    rare repeated-label positions].  Scans are right-trimmed to the frames
    from which the terminal states remain reachable.  ~300 DVE ops total;
    scan state is fp32 internally.

  Loss partials summed on host (no collectives needed).
"""

import numpy as np

import concourse.bass as bass
import concourse.bacc as bacc
import concourse.tile as tile
import concourse.mybir as mybir
from concourse.bass_utils import run_bass_kernel_spmd

BF16 = mybir.dt.bfloat16
F32 = mybir.dt.float32
FP8 = mybir.dt.float8e4
AF = mybir.ActivationFunctionType
ALU = mybir.AluOpType
AX = mybir.AxisListType
DR = mybir.MatmulPerfMode.DoubleRow

# Problem shapes (hardcoded per spec nn_CTC_28819230556189)
B, T, E, V, L = 32, 500, 1024, 5000, 100
S = 2 * L + 1           # 201 extended labels
NCORE = 8
BPC = B // NCORE        # 4 examples per core
NPAIR = E // 256        # 4 double-row K-pairs (256 contraction each)
TC = 125                # time chunk
NCHUNK = T // TC        # 4
VC = 500                # v-chunk width (one PSUM bank in f32)
NV = V // VC            # 10
C_SHIFT = 4.0           # logsumexp constant shift (logits ~ N(0,1))
D_SHIFT = -1.1          # p~ = exp(glog + D); keeps ln(sum B~) drift ~ 0
NR = L + 1              # distinct p~ rows per example (blank + labels)
TCP = 128               # hp stationary pair-stride (16B-aligned pad of TC)
RP = 112                # wext stationary pair-stride (16B-aligned pad of NR)

_cache = {}


def _hi(s):
    """Last frame (inclusive) from which state s can still reach a terminal
    state ({S-2, S-1}) by frame T-1; B_s beyond it cannot contribute."""
    need = max(0, (S - 2) - s)
    return min(T - 1, T - 1 - (need + 1) // 2)


def _build_nc(masked_odd):
    """masked_odd: sorted tuple of odd s positions where some example in the
    batch has a repeated label (skip transition disallowed) -> those sweep
    iterations apply the per-example m2 mask; all other odd s use a plain
    add (mask == 1 for every example by construction)."""
    nc = bacc.Bacc("TRN2", target_bir_lowering=False, debug=False,
                   enable_asserts=False)

    for val in (-C_SHIFT, D_SHIFT):
        cth = nc.alloc_sbuf_tensor(f"const-f32-{val}", [128, 1], F32)
        nc.gpsimd.memset(cth.ap(), val)
        nc.const_aps.aps[(F32, val)] = cth.ap()
    nc.all_engine_barrier()

    hpt_d = nc.dram_tensor("hpt", [BPC, NPAIR, 2, 128, T], FP8,
                           kind="ExternalInput")
    wtt_d = nc.dram_tensor("wtt", [NPAIR, 2, 128, V], FP8,
                           kind="ExternalInput")
    wxt_d = nc.dram_tensor("wxt", [BPC, NPAIR, 2, 128, NR], FP8,
                           kind="ExternalInput")
    m2_d = nc.dram_tensor("m2", [BPC, S], F32, kind="ExternalInput")
    out_d = nc.dram_tensor("out", [1, 1], F32, kind="ExternalOutput")

    with tile.TileContext(nc) as tc:
      with tc.tile_pool(name="persist", bufs=1) as pers:
        def ptile(shape, dtype, nm):
            return pers.tile(shape, dtype, tag=nm, name=nm)

        # ---- resident weights ----
        wt_all = ptile([128, NPAIR, 2, V], FP8, "wt_all")
        for pe in range(NPAIR):
            for i in range(2):
                nc.sync.dma_start(wt_all[:, pe, i, :], wtt_d[pe, i])
        wx_all = ptile([128, BPC, NPAIR, 2, RP], FP8, "wx_all")
        for bb in range(BPC):
            for pe in range(NPAIR):
                for i in range(2):
                    nc.sync.dma_start(wx_all[:, bb, pe, i, 0:NR],
                                      wxt_d[bb, pe, i])
        m2t = ptile([BPC, S], F32, "m2t")
        nc.sync.dma_start(m2t[:], m2_d[:])
        ones125 = ptile([125, 1], BF16, "ones125")
        nc.vector.memset(ones125[:], 1.0)

        # ---- sweep state ----
        psweep = ptile([BPC, NR, T], BF16, "psweep")
        brows = ptile([BPC, 3, T + 1], BF16, "brows")
        nc.vector.memset(brows[:], 0.0)
        nc.vector.memset(brows[:, 0, 0:1], 1.0)   # B_0(-1) = 1
        zrow = ptile([BPC, T], BF16, "zrow")
        nc.vector.memset(zrow[:], 0.0)
        vtmp = ptile([BPC, T], BF16, "vtmp")
        llacc = ptile([1, BPC], F32, "llacc")     # Sum_t ln lsum_t per ex
        nc.vector.memset(llacc[:], 0.0)

        with (
            tc.tile_pool(name="hp", bufs=2) as hp_pool,
            tc.tile_pool(name="scr", bufs=2) as scr_pool,
            tc.tile_pool(name="small", bufs=4) as small_pool,
            tc.tile_pool(name="pt", bufs=2) as pt_pool,
            tc.tile_pool(name="ps", bufs=2, space="PSUM") as ps_pool,
            tc.tile_pool(name="gl", bufs=2, space="PSUM") as gl_pool,
            tc.tile_pool(name="lsps", bufs=2, space="PSUM") as lsps_pool,
        ):
            # ================= Phase 1: matmuls / lse / p~ =================
            for c in range(NCHUNK):
                t0 = c * TC
                for bb in range(BPC):
                    hp_t = hp_pool.tile([128, NPAIR, 2, TCP], FP8, tag="hp",
                                        name="hp_t")
                    for pe in range(NPAIR):
                        for i in range(2):
                            nc.sync.dma_start(
                                hp_t[:, pe, i, 0:TC],
                                hpt_d[bb, pe, i, :, t0:t0 + TC])

                    spart = small_pool.tile([TC, NV // 2], F32, tag="spart",
                                            name="spart")
                    for k in range(NV // 2):
                        # two v-chunks -> two PSUM banks, one paired Act op
                        ps = ps_pool.tile([TC, 2, 512], F32, tag="ps",
                                          name="ps")
                        for h in range(2):
                            v = 2 * k + h
                            for pe in range(NPAIR):
                                nc.tensor.matmul(
                                    ps[:, h, 0:VC],
                                    hp_t[:, pe, :, 0:TC],
                                    wt_all[:, pe, :, v * VC:(v + 1) * VC],
                                    start=(pe == 0), stop=(pe == NPAIR - 1),
                                    perf_mode=DR)
                        scr = scr_pool.tile([TC, 2, VC], BF16, tag="scr",
                                            name="scr")
                        nc.scalar.activation(scr[:], ps[:, :, 0:VC], AF.Exp,
                                             bias=-C_SHIFT, scale=1.0,
                                             accum_out=spart[:, k:k + 1])
                    scr10 = small_pool.tile([TC, NV // 2], BF16, tag="scr10",
                                            name="scr10")
                    lsum = small_pool.tile([TC, 1], F32, tag="lsum",
                                           name="lsum")
                    nc.scalar.activation(scr10[:], spart[:], AF.Identity,
                                         accum_out=lsum[:])
                    lnls = small_pool.tile([TC, 1], BF16, tag="lnls",
                                           name="lnls")
                    nc.scalar.activation(lnls[:], lsum[:], AF.Ln)
                    # Sum_t ln lsum via ones-matmul partition reduce
                    lsps = lsps_pool.tile([1, 1], F32, tag="lsps",
                                          name="lsps")
                    nc.tensor.matmul(lsps[:], ones125[:], lnls[:],
                                     start=True, stop=True)
                    nc.vector.tensor_add(llacc[:, bb:bb + 1],
                                         llacc[:, bb:bb + 1], lsps[:])

                    # glog^T [token-row, t], then p~ = exp(glog + D) in bf16
                    gl = gl_pool.tile([NR, TC], F32, tag="gl", name="gl")
                    for pe in range(NPAIR):
                        nc.tensor.matmul(
                            gl[:], wx_all[:, bb, pe, :, 0:NR],
                            hp_t[:, pe, :, 0:TC],
                            start=(pe == 0), stop=(pe == NPAIR - 1),
                            perf_mode=DR)
                    ptc = pt_pool.tile([NR, TC], BF16, tag="ptc", name="ptc")
                    nc.scalar.activation(ptc[:], gl[:], AF.Exp,
                                         bias=D_SHIFT, scale=1.0)
                    nc.sync.dma_start(psweep[bb:bb + 1, :, t0:t0 + TC],
                                      ptc[:])

            # ================= Phase 2: s-sweep DP =================
            masked = set(masked_odd)
            for s in range(S):
                hi = _hi(s)
                w = hi + 1                      # frames [0, hi]
                row = brows[:, s % 3, :]
                out_ap = row[:, 1:w + 1]
                prow = psweep[:, 0 if s % 2 == 0 else 1 + (s - 1) // 2, 0:w]
                if s == 3:
                    # B_0(-1)=1 was consumed by s=1; row 0 now recycles as
                    # B_3 whose halo must read 0 for s=4/s=5.
                    nc.vector.memset(brows[:, 0, 0:1], 0.0)
                if s == 0:
                    nc.vector.tensor_tensor_scan(
                        out_ap, zrow[:, 0:w], prow, 1.0, ALU.add, ALU.mult)
                    continue
                b1 = brows[:, (s - 1) % 3, 0:w]
                if s == 1 or s % 2 == 0:
                    # v = B_{s-1}(t-1) only (blank, or s=1 which has no s-2)
                    nc.vector.tensor_tensor_scan(
                        out_ap, b1, prow, 0.0, ALU.add, ALU.mult)
                    continue
                b2 = brows[:, (s - 2) % 3, 0:w]
                if s in masked:
                    nc.vector.tensor_scalar_mul(vtmp[:, 0:w], b2,
                                                m2t[:, s:s + 1])
                    nc.vector.tensor_add(vtmp[:, 0:w], vtmp[:, 0:w], b1)
                else:
                    nc.vector.tensor_add(vtmp[:, 0:w], b1, b2)
                nc.vector.tensor_tensor_scan(
                    out_ap, vtmp[:, 0:w], prow, 0.0, ALU.add, ALU.mult)

            # ================= finalize =================
            u = ptile([BPC, 1], F32, "u")
            nc.vector.tensor_add(u[:], brows[:, 200 % 3, T:T + 1],
                                 brows[:, 199 % 3, T:T + 1])
            lnu = ptile([BPC, 1], F32, "lnu")
            nc.scalar.activation(lnu[:], u[:], AF.Ln)
            llf = ptile([1, BPC], F32, "llf")
            nc.sync.dma_start(llf[:], lnu[:])   # [4,1] -> [1,4]
            dif = ptile([1, BPC], F32, "dif")
            nc.vector.tensor_tensor(dif[:], llf[:], llacc[:], ALU.subtract)
            tot = ptile([1, 1], F32, "tot")
            nc.vector.tensor_reduce(tot[:], dif[:], axis=AX.X, op=ALU.add)
            nc.sync.dma_start(out_d[:], tot[:])

    nc.compile()
    return nc


def prep_in_maps(hpad, W, b, ys):
    """Host-side layout prep shared by kernel() and test harnesses."""
    f8 = mybir.dt.np(FP8)
    W = np.asarray(W)
    ext = np.zeros((B, S), dtype=np.int64)
    ext[:, 1::2] = ys
    prev2 = np.full((B, S), -1, dtype=np.int64)
    prev2[:, 2:] = ext[:, :-2]
    allow2 = (ext != 0) & (ext != prev2)
    masked_odd = tuple(sorted(
        s for s in range(3, S, 2) if not allow2[:, s].all()))
    m2 = allow2.astype(np.float32)

    hpT = np.ascontiguousarray(hpad.transpose(0, 2, 1)).astype(f8)
    hpT = hpT.reshape(B, NPAIR, 2, 128, T)
    wtT = np.ascontiguousarray(W.T).astype(f8).reshape(NPAIR, 2, 128, V)
    # distinct tokens per example: row 0 = blank, row 1+j = label j
    toks = np.concatenate([np.zeros((B, 1), np.int64), np.asarray(ys, np.int64)],
                          axis=1)                       # [B, NR]
    wext = np.ascontiguousarray(
        W[toks.reshape(-1)].reshape(B, NR, E).transpose(0, 2, 1)
    ).astype(f8).reshape(B, NPAIR, 2, 128, NR)

    in_maps = []
    for c in range(NCORE):
        sl = slice(c * BPC, (c + 1) * BPC)
        in_maps.append({
            "hpt": np.ascontiguousarray(hpT[sl]),
            "wtt": wtT,
            "wxt": np.ascontiguousarray(wext[sl]),
            "m2": np.ascontiguousarray(m2[sl]),
        })
    return in_maps, masked_odd


def kernel(hpad, W, b, ys):
    assert hpad.shape == (B, T, E) and W.shape == (V, E) and ys.shape == (B, L)
    assert not np.any(np.asarray(b)), "kernel assumes b == 0 (per problem spec)"

    in_maps, masked_odd = prep_in_maps(hpad, W, b, ys)
    key = ("nc", masked_odd)
    if key not in _cache:
        _cache[key] = _build_nc(masked_odd)
    nc = _cache[key]
    _cache["nc_last"] = (nc, in_maps)

    res = run_bass_kernel_spmd(nc, in_maps, core_ids=list(range(NCORE)))
    tot = sum(float(r["out"][0, 0]) for r in res.results)
    ll_sum = tot - B * T * (C_SHIFT + D_SHIFT)
    return np.float32(-ll_sum / B)
